# revision 43
# baseline (speedup 1.0000x reference)
"""Trainium2 Bass kernel for nn_MergeZoom: per-sample mask bbox + crop + bilinear resize.

Algorithm (per sample, all on-device):
  mb   = (mask >= 0.5)
  rows/cols nonzero -> bbox (first,last per axis) via exact count/weighted-sum trick
  out  = R @ (mb * image) @ C^T  where R/C are bilinear "tent" matrices built on-chip.
  Tents are built NEGATED (min(|src-p|,1) - 1 = -relu(1-|src-p|)); the sign cancels
  across the two interpolation matmul stages.

Perf notes (176us -> ~136us):
  - PE warmup bursts at kernel start keep the HAM clock-gate at 2.4 GHz (cold MMs
    run at 1.2 GHz: 427ns vs 216ns for an N=512 bf16 matmul).
  - Masked image stored planar [p, (c, t, w)] so stage-1 lhsT slabs are contiguous
    (enables FWL weight-load overlap; strided weights serialize LDW with the MM).
  - Software-pipelined across samples: preprocessing of sample s+1 (mask stats on
    Scalar w/ accum_out, bbox chain batched [128,2] on DVE, tents split
    Scalar(A)/DVE(B)) overlaps the interp matmuls of sample s on PE.
  - Row tents positive (Scalar Relu), col tents negated (DVE min-sub); the sign is
    fixed for free in the stage-2 PSUM evacuation (scale=-1).
  - PSUM: both pools' rings are shared by both stages (7 tiles in flight) to hide
    evacuation latency; output DMA streams per 128-row chunk.
  - GpSimd deliberately unused: its tensor ops are ~10-20x slower than DVE and
    contend for the shared SBUF port.

Sharding: pure data-parallel, 4 samples per core across 8 cores.
"""

import numpy as np

import concourse.bass as bass
import concourse.tile as tile
from concourse import bacc, mybir

B = 32
N_CORES = 8
BPC = B // N_CORES  # samples per core
H = W = 512
C = 3
HT = H // 128  # 4 h-chunks of 128 partitions
WT = W // 128

FP = mybir.dt.float32
BF = mybir.dt.bfloat16
AX = mybir.AxisListType.X
OP = mybir.AluOpType
AF = mybir.ActivationFunctionType

N_WARMUP = 40


def build(bpc: int = BPC) -> bass.Bass:
    nc = bacc.Bacc()
    mask_d = nc.declare_dram_parameter("mask", [bpc, H, W, 1], FP, isOutput=False)
    img_d = nc.declare_dram_parameter("image", [bpc, H, W, C], FP, isOutput=False)
    iota_d = nc.declare_dram_parameter("iota_f", [128, 512], FP, isOutput=False)
    pidx_d = nc.declare_dram_parameter("pidx", [128, HT], FP, isOutput=False)
    tp_d = nc.declare_dram_parameter("tp_h", [128, 3 * HT], BF, isOutput=False)
    out_d = nc.declare_dram_parameter("out", [bpc, H, W, C], FP, isOutput=True)

    with tile.TileContext(nc) as tc:
        with (
            tc.tile_pool(name="consts", bufs=1) as cpool,
            tc.tile_pool(name="io", bufs=2) as iopool,
            tc.tile_pool(name="work", bufs=1) as wk,
            tc.tile_pool(name="small", bufs=2) as sm,
            tc.tile_pool(name="ps1", bufs=4, space="PSUM") as ps1p,
            tc.tile_pool(name="ps2", bufs=3, space="PSUM") as ps2p,
        ):
            iota = cpool.tile([128, 512], FP)
            nc.sync.dma_start(iota[:], iota_d[:])
            pidx = cpool.tile([128, HT], FP)
            nc.sync.dma_start(pidx[:], pidx_d[:])
            tp = cpool.tile([128, 3 * HT], BF)
            nc.sync.dma_start(tp[:], tp_d[:])
            onesh = cpool.tile([128, 128], BF)
            nc.vector.memset(onesh[:], 1.0)
            warm_rhs = cpool.tile([128, 512], BF)
            nc.vector.memset(warm_rhs[:], 0.001)
            negp = cpool.tile([128, HT], FP)
            nc.vector.tensor_scalar(negp[:], pidx[:], -1.0, None, OP.mult)
            neghalf = cpool.tile([128, 1], FP)
            nc.vector.memset(neghalf[:], -0.5)

            # persistent tiles (explicit double-buffer via tags)
            def t_msk(s):
                return sm.tile([128, HT * 512], FP, tag="msk", bufs=1, name="msk")

            def t_img(s):
                return iopool.tile([128, HT * 512 * C], FP, tag="img", bufs=2, name="img")

            def t_outt(s):
                return iopool.tile([128, HT * 512 * C], FP, tag="outt", bufs=2, name="outt")

            def t_mbh(s):
                return sm.tile([128, HT * 512], BF, tag="mbh", bufs=1, name="mbh")

            def t_Mh(s):
                return wk.tile([128, C * HT * 512], BF, tag="Mh", bufs=2, name="Mh")

            def t_t1(s):
                return wk.tile([128, C * WT * 512], BF, tag="t1", bufs=2, name="t1")

            def t_RT(s):
                return wk.tile([128, HT * 512], BF, tag="RT", bufs=2, name="RT")

            def t_CT(s):
                return wk.tile([128, WT * 512], BF, tag="CT", bufs=2, name="CT")

            # small per-sample state
            def t_r4(s):
                return sm.tile([128, HT], FP, tag="r4", bufs=2, name="r4")

            def t_NS(s):
                return sm.tile([128, 8], FP, tag="NS", bufs=2, name="NS")

            def t_sc(s):
                return sm.tile([128, 16], FP, tag="sc", bufs=2, name="sc")

            def t_src(s, o):
                return sm.tile([128, 512], FP, tag=f"src{o}", bufs=2, name=f"src{o}")

            # tile-state caches so helper phases share handles per sample
            state: dict = {}

            def st(s, key, mk):
                k = (s, key)
                if k not in state:
                    state[k] = mk(s)
                return state[k]

            # ---------------- warmup: keep PE HAM clock-gate open ----------------
            def warm_burst(n):
                warm_ps = ps2p.tile([128, 512], FP, tag="pscols", name="pscols", bufs=1)
                for _ in range(n):
                    nc.tensor.matmul(
                        warm_ps[:], onesh[:], warm_rhs[:], start=True, stop=True
                    )

            warm_burst(12)

            # ---------------- emission helpers ----------------
            def pre_load(s):
                msk = st(s, "msk", t_msk)
                mskd = mask_d[s].rearrange("(t p) w one -> t p (w one)", p=128)
                for t in range(HT):
                    nc.sync.dma_start(
                        msk[:, t * 512 : (t + 1) * 512], mskd[t]
                    )
                img = st(s, "img", t_img)
                nc.sync.dma_start(
                    img[:].rearrange("p (t x) -> p t x", t=HT),
                    img_d[s]
                    .rearrange("(t p) w c -> t p (w c)", p=128)
                    .transpose([1, 0, 2]),
                )

            def pre_early(s):
                # binarize + row-sum accum per chunk (DVE); col-sum MMs (PE)
                msk = st(s, "msk", t_msk)
                mbh = st(s, "mbh", t_mbh)
                r4 = st(s, "r4", t_r4)
                pscols = ps2p.tile([128, 512], FP, tag="pscols", name="pscols", bufs=1)
                state[(s, "pscols")] = pscols
                for t in range(HT):
                    nc.vector.scalar_tensor_tensor(
                        mbh[:, t * 512 : (t + 1) * 512],
                        msk[:, t * 512 : (t + 1) * 512],
                        0.5, msk[:, t * 512 : (t + 1) * 512],
                        OP.is_ge, OP.mult,
                        accum_out=r4[:, t : t + 1],
                    )
                    nc.tensor.matmul(
                        pscols[:],
                        onesh[:],
                        mbh[:, t * 512 : (t + 1) * 512],
                        start=(t == 0),
                        stop=(t == HT - 1),
                    )
                rwh = sm.tile([128, 3 * HT], BF, tag="rwh", bufs=2, name="rwh")
                state[(s, "rwh")] = rwh
                nc.vector.tensor_scalar(rwh[:, 0:HT], r4[:], 0.0, None, OP.is_gt)
                nc.vector.tensor_tensor(
                    rwh[:, HT : 2 * HT], rwh[:, 0:HT], tp[:, HT : 2 * HT], OP.mult
                )
                nc.vector.tensor_tensor(
                    rwh[:, 2 * HT : 3 * HT], rwh[:, 0:HT], tp[:, 2 * HT : 3 * HT], OP.mult
                )


            def pre_mid(s):
                # bbox stats: NS = [Nr, Nc, Sr(_t), Sc, Sr_p]
                pscols = state[(s, "pscols")]
                NS = st(s, "NS", t_NS)
                colscr = sm.tile([128, 512], FP, tag="colscr", bufs=1, name="colscr")
                redscr = sm.tile([128, 3 * HT], FP, tag="redscr", bufs=2, name="redscr")
                nc.scalar.activation(
                    colscr[:], pscols[:], AF.Sign, accum_out=NS[:, 1:2]
                )
                colt = sm.tile([128, 512], FP, tag="colt", bufs=2, name="colt")
                nc.vector.scalar_tensor_tensor(
                    colt[:], pscols[:], 0.0, iota[:], OP.is_gt, OP.mult,
                    accum_out=NS[:, 3:4],
                )
                rwh = state[(s, "rwh")]
                nc.tensor.matmul(
                    pscols[:, 0 : 3 * HT], onesh[:], rwh[:], start=True, stop=True,
                    skip_group_check=True,
                )
                nc.scalar.activation(
                    redscr[:, 0:HT], pscols[:, 0:HT], AF.Copy,
                    accum_out=NS[:, 0:1],
                )
                nc.scalar.activation(
                    redscr[:, HT : 2 * HT], pscols[:, HT : 2 * HT], AF.Copy,
                    accum_out=NS[:, 2:3],
                )
                nc.scalar.activation(
                    redscr[:, 2 * HT : 3 * HT], pscols[:, 2 * HT : 3 * HT], AF.Copy,
                    accum_out=NS[:, 4:5],
                )
                # Sr = 128*Sr_t + Sr_p
                nc.scalar.activation(
                    NS[:, 2:3], NS[:, 2:3], AF.Identity, scale=128.0, bias=NS[:, 4:5]
                )
                # batched bbox chain over both axes: [Nr, Nc] = NS[:,0:2], [Sr, Sc] = NS[:,2:4]
                # sc: 0:2 recip | 2:4 mean | 4:6 hw | 6:8 first | 8:10 last(hi1)
                #     | 10:12 a | 12:14 b | 14:16 lo
                sc = st(s, "sc", t_sc)
                nc.vector.reciprocal(sc[:, 0:2], NS[:, 0:2])
                nc.vector.tensor_tensor(sc[:, 2:4], NS[:, 2:4], sc[:, 0:2], OP.mult)
                nc.vector.tensor_scalar(
                    sc[:, 4:6], NS[:, 0:2], -1.0, 0.5, OP.add, OP.mult
                )
                nc.vector.tensor_tensor(sc[:, 6:8], sc[:, 2:4], sc[:, 4:6], OP.subtract)
                nc.vector.tensor_tensor(sc[:, 8:10], sc[:, 2:4], sc[:, 4:6], OP.add)
                nc.vector.tensor_scalar(
                    sc[:, 10:12], NS[:, 0:2], 1.0, 1.0 / 512.0, OP.add, OP.mult
                )
                nc.vector.tensor_scalar(
                    sc[:, 12:14], sc[:, 10:12], 0.5, -1.5, OP.mult, OP.add
                )
                nc.vector.tensor_tensor(sc[:, 12:14], sc[:, 12:14], sc[:, 6:8], OP.add)
                nc.vector.tensor_scalar(sc[:, 14:16], sc[:, 6:8], -1.0, None, OP.add)

            def pre_late(s, which=(0, 1)):
                sc = st(s, "sc", t_sc)
                axes = [(0, "RT", t_RT), (1, "CT", t_CT)]
                for o, key, mk in (axes[i] for i in which):
                    mat = st(s, key, mk)
                    src = st(s, f"src{o}", lambda s_: t_src(s_, o))
                    nc.vector.tensor_scalar(
                        src[:], iota[:], sc[:, 10 + o : 11 + o], sc[:, 12 + o : 13 + o],
                        OP.mult, OP.add,
                    )
                    nc.vector.tensor_scalar(
                        src[:], src[:], sc[:, 14 + o : 15 + o], sc[:, 8 + o : 9 + o],
                        OP.max, OP.min,
                    )
                    tmp4 = sm.tile(
                        [128, HT * 512], BF, tag=f"tent{o}", bufs=2, name=f"tent{o}"
                    )
                    for t in range(HT):
                        nc.scalar.activation(
                            tmp4[:, t * 512 : (t + 1) * 512], src[:], AF.Abs,
                            bias=negp[:, t : t + 1], scale=1.0,
                        )
                        if o == 0:
                            nc.vector.tensor_scalar(
                                mat[:, t * 512 : (t + 1) * 512],
                                tmp4[:, t * 512 : (t + 1) * 512],
                                1.0, 1.0, OP.min, OP.subtract,
                            )
                    if o == 1:
                        nc.vector.tensor_scalar(
                            mat[:], tmp4[:], 1.0, 1.0, OP.min, OP.subtract
                        )

            def pre_mh(s, chans=(0, 1, 2), thalf=None):
                # masked image, planar [p, (c, t, w)] (DVE, fused binarize+mult)
                msk = st(s, "msk", t_msk)
                img = st(s, "img", t_img)
                Mh = st(s, "Mh", t_Mh)
                img4 = img[:].rearrange("p (t w c) -> p t w c", t=HT, w=512)
                Mh4 = Mh[:].rearrange("p (c t w) -> p c t w", c=C, t=HT)
                msk3 = msk[:].rearrange("p (t w) -> p t w", t=HT)
                tsl = slice(None) if thalf is None else slice(
                    thalf * (HT // 2), (thalf + 1) * (HT // 2)
                )
                for c in chans:
                    nc.vector.scalar_tensor_tensor(
                        Mh4[:, c, tsl], msk3[:, tsl], 0.5, img4[:, tsl, :, c],
                        OP.is_ge, OP.mult,
                    )

            evac_cp = [0]

            def evac(dst, ps, negate=False, dve_mod=3):
                if evac_cp[0] % dve_mod == 0:
                    if negate:
                        nc.vector.tensor_scalar(dst, ps, -1.0, None, OP.mult)
                    else:
                        nc.vector.tensor_copy(dst, ps)
                else:
                    if negate:
                        nc.scalar.activation(dst, ps, AF.Identity, scale=-1.0)
                    else:
                        nc.scalar.copy(dst, ps)
                evac_cp[0] += 1

            def stage1_group(s, c, wt, pool=None):
                Mh = st(s, "Mh", t_Mh)
                RT = st(s, "RT", t_RT)
                t1 = st(s, "t1", t_t1)
                Mh4 = Mh[:].rearrange("p (c t w) -> p c t w", c=C, t=HT)
                if pool is None:
                    ps1 = ps1p.tile([128, 512], FP, tag="ps1", name="ps1")
                else:
                    ps1 = pool.tile([128, 512], FP, tag="ps2", name="ps2")
                for ht in range(HT):
                    lhsT = Mh4[:, c, ht, wt * 128 : (wt + 1) * 128]
                    nc.tensor.matmul(
                        ps1[:],
                        lhsT,
                        RT[:, ht * 512 : (ht + 1) * 512],
                        start=(ht == 0),
                        stop=(ht == HT - 1),
                    )
                dst = t1[:, (c * WT + wt) * 512 : (c * WT + wt + 1) * 512]
                evac(dst, ps1[:])

            def stage2_group(s, c, ot, out4, pool=None):
                t1 = st(s, "t1", t_t1)
                CT = st(s, "CT", t_CT)
                if pool is None:
                    ps2 = ps2p.tile([128, 512], FP, tag="ps2", name="ps2")
                else:
                    ps2 = pool.tile([128, 512], FP, tag="ps1", name="ps1")
                for wt in range(WT):
                    lhsT2 = t1[
                        :,
                        (c * WT + wt) * 512 + ot * 128 : (c * WT + wt) * 512
                        + (ot + 1) * 128,
                    ]
                    nc.tensor.matmul(
                        ps2[:],
                        lhsT2,
                        CT[:, wt * 512 : (wt + 1) * 512],
                        start=(wt == 0),
                        stop=(wt == WT - 1),
                    )
                evac(out4[:, ot, :, c], ps2[:], dve_mod=2)

            def emit_out(s):
                outt = state[(s, "outt")]
                nc.sync.dma_start(
                    out_d[s]
                    .rearrange("(t p) w c -> t p (w c)", p=128)
                    .transpose([1, 0, 2]),
                    outt[:].rearrange("p (t x) -> p t x", t=HT),
                )

            # ---------------- software-pipelined emission ----------------
            pre_load(0)
            warm_burst(10)
            pre_early(0)
            pre_mid(0)
            warm_burst(10)
            pre_late(0)
            warm_burst(10)
            pre_mh(0)
            warm_burst(20)

            for s in range(bpc):
                nxt = s + 1 if s + 1 < bpc else None
                if nxt is not None:
                    pre_load(nxt)

                outt = st(s, "outt", t_outt)
                out4 = outt[:].rearrange("p (t w c) -> p t w c", t=HT, w=512)

                g1 = [(c, wt) for c in range(C) for wt in range(WT)]
                for i, (c, wt) in enumerate(g1):
                    stage1_group(s, c, wt, pool=(None if i % 2 == 0 else ps2p))
                    if i == 1 and nxt is not None:
                        pre_early(nxt)
                    if i == 4 and nxt is not None:
                        pre_mid(nxt)
                    if i == 6 and nxt is not None:
                        pre_late(nxt, which=(0,))

                outd3 = out_d[s].rearrange("(t p) w c -> t p (w c)", p=128)
                g2 = [(ot, c) for ot in range(HT) for c in range(C)]
                for i, (ot, c) in enumerate(g2):
                    stage2_group(s, c, ot, out4, pool=(None if i % 2 == 0 else ps1p))
                    if i == 0 and nxt is not None:
                        pre_late(nxt, which=(1,))
                    if i == 2 and nxt is not None:
                        pre_mh(nxt, chans=(0,))
                    if i == 4 and nxt is not None:
                        pre_mh(nxt, chans=(1,))
                    if i == 6 and nxt is not None:
                        pre_mh(nxt, chans=(2,))
                    if i % C == C - 1:
                        nc.sync.dma_start(
                            outd3[ot],
                            outt[:, ot * 512 * C : (ot + 1) * 512 * C],
                        )

    nc.compile()
    return nc


def make_consts() -> dict[str, np.ndarray]:
    import ml_dtypes

    iota_f = np.broadcast_to(np.arange(512, dtype=np.float32), (128, 512)).copy()
    p = np.arange(128, dtype=np.float32)
    pidx = np.stack([p + 128 * t for t in range(HT)], axis=1).astype(np.float32)
    tvals = np.broadcast_to(
        np.arange(HT, dtype=np.float32)[None, :], (128, HT)
    ).astype(np.float32)
    pvals = np.broadcast_to(p[:, None], (128, HT)).astype(np.float32)
    ones4 = np.ones((128, HT), np.float32)
    tp_h = np.concatenate([ones4, tvals, pvals], axis=1).astype(ml_dtypes.bfloat16)
    return {"iota_f": iota_f, "pidx": pidx, "tp_h": tp_h}


_NC_CACHE: dict[int, bass.Bass] = {}


def _get_nc(bpc: int = BPC) -> bass.Bass:
    if bpc not in _NC_CACHE:
        _NC_CACHE[bpc] = build(bpc)
    return _NC_CACHE[bpc]


def run(mask: np.ndarray, image: np.ndarray, trace: bool = False, **kwargs):
    """Run on 8 cores; returns (out [B,H,W,C], BassKernelResults)."""
    from concourse.bass_utils import run_bass_kernel_spmd

    nc = _get_nc(BPC)
    consts = make_consts()
    mask = np.ascontiguousarray(mask, dtype=np.float32)
    image = np.ascontiguousarray(image, dtype=np.float32)
    in_maps = []
    for i in range(N_CORES):
        m = {
            "mask": mask[i * BPC : (i + 1) * BPC],
            "image": image[i * BPC : (i + 1) * BPC],
        }
        m.update(consts)
        in_maps.append(m)
    res = run_bass_kernel_spmd(nc, in_maps, list(range(N_CORES)), trace=trace, **kwargs)
    out = np.concatenate([res.results[i]["out"] for i in range(N_CORES)], axis=0)
    return out, res


def kernel(mask: np.ndarray, image: np.ndarray) -> np.ndarray:
    out, _ = run(mask, image)
    return out.astype(np.float32)


# revision 44
# speedup vs baseline: 1.0266x; 1.0266x over previous
"""Trainium2 Bass kernel for nn_MergeZoom: per-sample mask bbox + crop + bilinear resize.

Algorithm (per sample, all on-device):
  mb   = (mask >= 0.5)
  rows/cols nonzero -> bbox (first,last per axis) via exact count/weighted-sum trick
  out  = R @ (mb * image) @ C^T  where R/C are bilinear "tent" matrices built on-chip.
  Tents are built NEGATED (min(|src-p|,1) - 1 = -relu(1-|src-p|)); the sign cancels
  across the two interpolation matmul stages.

Perf notes (176us -> ~136us):
  - PE warmup bursts at kernel start keep the HAM clock-gate at 2.4 GHz (cold MMs
    run at 1.2 GHz: 427ns vs 216ns for an N=512 bf16 matmul).
  - Masked image stored planar [p, (c, t, w)] so stage-1 lhsT slabs are contiguous
    (enables FWL weight-load overlap; strided weights serialize LDW with the MM).
  - Software-pipelined across samples: preprocessing of sample s+1 (mask stats on
    Scalar w/ accum_out, bbox chain batched [128,2] on DVE, tents split
    Scalar(A)/DVE(B)) overlaps the interp matmuls of sample s on PE.
  - Row tents positive (Scalar Relu), col tents negated (DVE min-sub); the sign is
    fixed for free in the stage-2 PSUM evacuation (scale=-1).
  - PSUM: both pools' rings are shared by both stages (7 tiles in flight) to hide
    evacuation latency; output DMA streams per 128-row chunk.
  - GpSimd deliberately unused: its tensor ops are ~10-20x slower than DVE and
    contend for the shared SBUF port.

Sharding: pure data-parallel, 4 samples per core across 8 cores.
"""

import numpy as np

import concourse.bass as bass
import concourse.tile as tile
from concourse import bacc, mybir

B = 32
N_CORES = 8
BPC = B // N_CORES  # samples per core
H = W = 512
C = 3
HT = H // 128  # 4 h-chunks of 128 partitions
WT = W // 128

FP = mybir.dt.float32
BF = mybir.dt.bfloat16
AX = mybir.AxisListType.X
OP = mybir.AluOpType
AF = mybir.ActivationFunctionType

N_WARMUP = 40


def build(bpc: int = BPC) -> bass.Bass:
    nc = bacc.Bacc()
    mask_d = nc.declare_dram_parameter("mask", [bpc, H, W, 1], FP, isOutput=False)
    img_d = nc.declare_dram_parameter("image", [bpc, H, W, C], FP, isOutput=False)
    iota_d = nc.declare_dram_parameter("iota_f", [128, 512], FP, isOutput=False)
    pidx_d = nc.declare_dram_parameter("pidx", [128, HT], FP, isOutput=False)
    tp_d = nc.declare_dram_parameter("tp_h", [128, 3 * HT], BF, isOutput=False)
    out_d = nc.declare_dram_parameter("out", [bpc, H, W, C], FP, isOutput=True)

    with tile.TileContext(nc) as tc:
        with (
            tc.tile_pool(name="consts", bufs=1) as cpool,
            tc.tile_pool(name="io", bufs=2) as iopool,
            tc.tile_pool(name="work", bufs=1) as wk,
            tc.tile_pool(name="small", bufs=2) as sm,
            tc.tile_pool(name="ps1", bufs=4, space="PSUM") as ps1p,
            tc.tile_pool(name="ps2", bufs=3, space="PSUM") as ps2p,
        ):
            iota = cpool.tile([128, 512], FP)
            nc.sync.dma_start(iota[:], iota_d[:])
            pidx = cpool.tile([128, HT], FP)
            nc.sync.dma_start(pidx[:], pidx_d[:])
            tp = cpool.tile([128, 3 * HT], BF)
            nc.sync.dma_start(tp[:], tp_d[:])
            onesh = cpool.tile([128, 128], BF)
            nc.vector.memset(onesh[:], 1.0)
            warm_rhs = cpool.tile([128, 512], BF)
            nc.vector.memset(warm_rhs[:], 0.001)
            negp = cpool.tile([128, HT], FP)
            nc.vector.tensor_scalar(negp[:], pidx[:], -1.0, None, OP.mult)
            neghalf = cpool.tile([128, 1], FP)
            nc.vector.memset(neghalf[:], -0.5)

            # persistent tiles (explicit double-buffer via tags)
            def t_msk(s):
                return sm.tile([128, HT * 512], FP, tag="msk", bufs=1, name="msk")

            def t_img(s):
                return iopool.tile([128, HT * 512 * C], FP, tag="img", bufs=2, name="img")

            def t_outt(s):
                return iopool.tile([128, HT * 512 * C], FP, tag="outt", bufs=2, name="outt")

            def t_mbh(s):
                return sm.tile([128, HT * 512], BF, tag="mbh", bufs=1, name="mbh")

            def t_Mh(s):
                return wk.tile([128, C * HT * 512], BF, tag="Mh", bufs=2, name="Mh")

            def t_t1(s):
                return wk.tile([128, C * WT * 512], BF, tag="t1", bufs=2, name="t1")

            def t_RT(s):
                return wk.tile([128, HT * 512], BF, tag="RT", bufs=2, name="RT")

            def t_CT(s):
                return wk.tile([128, WT * 512], BF, tag="CT", bufs=2, name="CT")

            # small per-sample state
            def t_r4(s):
                return sm.tile([128, HT], FP, tag="r4", bufs=2, name="r4")

            def t_NS(s):
                return sm.tile([128, 8], FP, tag="NS", bufs=2, name="NS")

            def t_sc(s):
                return sm.tile([128, 16], FP, tag="sc", bufs=2, name="sc")

            def t_src(s, o):
                return sm.tile([128, 512], FP, tag=f"src{o}", bufs=2, name=f"src{o}")

            # tile-state caches so helper phases share handles per sample
            state: dict = {}

            def st(s, key, mk):
                k = (s, key)
                if k not in state:
                    state[k] = mk(s)
                return state[k]

            # ---------------- warmup: keep PE HAM clock-gate open ----------------
            def warm_burst(n):
                warm_ps = ps2p.tile([128, 512], FP, tag="pscols", name="pscols", bufs=1)
                for _ in range(n):
                    nc.tensor.matmul(
                        warm_ps[:], onesh[:], warm_rhs[:], start=True, stop=True
                    )

            warm_burst(12)

            # ---------------- emission helpers ----------------
            def pre_load(s):
                msk = st(s, "msk", t_msk)
                mskd = mask_d[s].rearrange("(t p) w one -> t p (w one)", p=128)
                for t in range(HT):
                    nc.sync.dma_start(
                        msk[:, t * 512 : (t + 1) * 512], mskd[t]
                    )
                img = st(s, "img", t_img)
                nc.sync.dma_start(
                    img[:].rearrange("p (t x) -> p t x", t=HT),
                    img_d[s]
                    .rearrange("(t p) w c -> t p (w c)", p=128)
                    .transpose([1, 0, 2]),
                )

            def pre_early(s):
                # binarize + row-sum accum per chunk (DVE); col-sum MMs (PE)
                msk = st(s, "msk", t_msk)
                mbh = st(s, "mbh", t_mbh)
                r4 = st(s, "r4", t_r4)
                pscols = ps2p.tile([128, 512], FP, tag="pscols", name="pscols", bufs=1)
                state[(s, "pscols")] = pscols
                for t in range(HT):
                    nc.vector.scalar_tensor_tensor(
                        mbh[:, t * 512 : (t + 1) * 512],
                        msk[:, t * 512 : (t + 1) * 512],
                        0.5, msk[:, t * 512 : (t + 1) * 512],
                        OP.is_ge, OP.mult,
                        accum_out=r4[:, t : t + 1],
                    )
                    nc.tensor.matmul(
                        pscols[:],
                        onesh[:],
                        mbh[:, t * 512 : (t + 1) * 512],
                        start=(t == 0),
                        stop=(t == HT - 1),
                    )
                rwh = sm.tile([128, 3 * HT], BF, tag="rwh", bufs=2, name="rwh")
                state[(s, "rwh")] = rwh
                nc.vector.tensor_scalar(rwh[:, 0:HT], r4[:], 0.0, None, OP.is_gt)
                nc.vector.tensor_tensor(
                    rwh[:, HT : 2 * HT], rwh[:, 0:HT], tp[:, HT : 2 * HT], OP.mult
                )
                nc.vector.tensor_tensor(
                    rwh[:, 2 * HT : 3 * HT], rwh[:, 0:HT], tp[:, 2 * HT : 3 * HT], OP.mult
                )


            def pre_mid(s):
                # bbox stats: NS = [Nr, Nc, Sr(_t), Sc, Sr_p]
                pscols = state[(s, "pscols")]
                NS = st(s, "NS", t_NS)
                colscr = sm.tile([128, 512], FP, tag="colscr", bufs=1, name="colscr")
                redscr = sm.tile([128, 3 * HT], FP, tag="redscr", bufs=2, name="redscr")
                nc.scalar.activation(
                    colscr[:], pscols[:], AF.Sign, accum_out=NS[:, 1:2]
                )
                colt = sm.tile([128, 512], FP, tag="colt", bufs=2, name="colt")
                nc.vector.scalar_tensor_tensor(
                    colt[:], pscols[:], 0.0, iota[:], OP.is_gt, OP.mult,
                    accum_out=NS[:, 3:4],
                )
                rwh = state[(s, "rwh")]
                nc.tensor.matmul(
                    pscols[:, 0 : 3 * HT], onesh[:], rwh[:], start=True, stop=True,
                    skip_group_check=True,
                )
                nc.scalar.activation(
                    redscr[:, 0:HT], pscols[:, 0:HT], AF.Copy,
                    accum_out=NS[:, 0:1],
                )
                nc.scalar.activation(
                    redscr[:, HT : 2 * HT], pscols[:, HT : 2 * HT], AF.Copy,
                    accum_out=NS[:, 2:3],
                )
                nc.scalar.activation(
                    redscr[:, 2 * HT : 3 * HT], pscols[:, 2 * HT : 3 * HT], AF.Copy,
                    accum_out=NS[:, 4:5],
                )
                # Sr = 128*Sr_t + Sr_p
                nc.scalar.activation(
                    NS[:, 2:3], NS[:, 2:3], AF.Identity, scale=128.0, bias=NS[:, 4:5]
                )
                # batched bbox chain over both axes: [Nr, Nc] = NS[:,0:2], [Sr, Sc] = NS[:,2:4]
                # sc: 0:2 recip | 2:4 mean | 4:6 hw | 6:8 first | 8:10 last(hi1)
                #     | 10:12 a | 12:14 b | 14:16 lo
                sc = st(s, "sc", t_sc)
                nc.vector.reciprocal(sc[:, 0:2], NS[:, 0:2])
                nc.vector.tensor_tensor(sc[:, 2:4], NS[:, 2:4], sc[:, 0:2], OP.mult)
                nc.vector.tensor_scalar(
                    sc[:, 4:6], NS[:, 0:2], -1.0, 0.5, OP.add, OP.mult
                )
                nc.vector.tensor_tensor(sc[:, 6:8], sc[:, 2:4], sc[:, 4:6], OP.subtract)
                nc.vector.tensor_tensor(sc[:, 8:10], sc[:, 2:4], sc[:, 4:6], OP.add)
                nc.vector.tensor_scalar(
                    sc[:, 10:12], NS[:, 0:2], 1.0, 1.0 / 512.0, OP.add, OP.mult
                )
                nc.vector.tensor_scalar(
                    sc[:, 12:14], sc[:, 10:12], 0.5, -1.5, OP.mult, OP.add
                )
                nc.vector.tensor_tensor(sc[:, 12:14], sc[:, 12:14], sc[:, 6:8], OP.add)
                nc.vector.tensor_scalar(sc[:, 14:16], sc[:, 6:8], -1.0, None, OP.add)

            def pre_late(s, which=(0, 1)):
                sc = st(s, "sc", t_sc)
                axes = [(0, "RT", t_RT), (1, "CT", t_CT)]
                for o, key, mk in (axes[i] for i in which):
                    mat = st(s, key, mk)
                    src = st(s, f"src{o}", lambda s_: t_src(s_, o))
                    nc.vector.tensor_scalar(
                        src[:], iota[:], sc[:, 10 + o : 11 + o], sc[:, 12 + o : 13 + o],
                        OP.mult, OP.add,
                    )
                    nc.vector.tensor_scalar(
                        src[:], src[:], sc[:, 14 + o : 15 + o], sc[:, 8 + o : 9 + o],
                        OP.max, OP.min,
                    )
                    tmp4 = sm.tile(
                        [128, HT * 512], BF, tag=f"tent{o}", bufs=2, name=f"tent{o}"
                    )
                    for t in range(HT):
                        nc.scalar.activation(
                            tmp4[:, t * 512 : (t + 1) * 512], src[:], AF.Abs,
                            bias=negp[:, t : t + 1], scale=1.0,
                        )
                        if o == 0:
                            nc.vector.tensor_scalar(
                                mat[:, t * 512 : (t + 1) * 512],
                                tmp4[:, t * 512 : (t + 1) * 512],
                                1.0, 1.0, OP.min, OP.subtract,
                            )
                    if o == 1:
                        nc.vector.tensor_scalar(
                            mat[:], tmp4[:], 1.0, 1.0, OP.min, OP.subtract
                        )

            def pre_mh(s, chans=(0, 1, 2), thalf=None):
                # masked image, planar [p, (c, t, w)] (DVE, fused binarize+mult)
                msk = st(s, "msk", t_msk)
                img = st(s, "img", t_img)
                Mh = st(s, "Mh", t_Mh)
                img4 = img[:].rearrange("p (t w c) -> p t w c", t=HT, w=512)
                Mh4 = Mh[:].rearrange("p (c t w) -> p c t w", c=C, t=HT)
                msk3 = msk[:].rearrange("p (t w) -> p t w", t=HT)
                tsl = slice(None) if thalf is None else slice(
                    thalf * (HT // 2), (thalf + 1) * (HT // 2)
                )
                for c in chans:
                    nc.vector.scalar_tensor_tensor(
                        Mh4[:, c, tsl], msk3[:, tsl], 0.5, img4[:, tsl, :, c],
                        OP.is_ge, OP.mult,
                    )

            evac_cp = [0]

            def evac(dst, ps, negate=False, dve_mod=3):
                if evac_cp[0] % dve_mod == 0:
                    if negate:
                        nc.vector.tensor_scalar(dst, ps, -1.0, None, OP.mult)
                    else:
                        nc.vector.tensor_copy(dst, ps)
                else:
                    if negate:
                        nc.scalar.activation(dst, ps, AF.Identity, scale=-1.0)
                    else:
                        nc.scalar.copy(dst, ps)
                evac_cp[0] += 1

            def stage1_group(s, c, wt, pool=None):
                Mh = st(s, "Mh", t_Mh)
                RT = st(s, "RT", t_RT)
                t1 = st(s, "t1", t_t1)
                Mh4 = Mh[:].rearrange("p (c t w) -> p c t w", c=C, t=HT)
                if pool is None:
                    ps1 = ps1p.tile([128, 512], FP, tag="ps1", name="ps1")
                else:
                    ps1 = pool.tile([128, 512], FP, tag="ps2", name="ps2")
                for ht in range(HT):
                    lhsT = Mh4[:, c, ht, wt * 128 : (wt + 1) * 128]
                    nc.tensor.matmul(
                        ps1[:],
                        lhsT,
                        RT[:, ht * 512 : (ht + 1) * 512],
                        start=(ht == 0),
                        stop=(ht == HT - 1),
                    )
                dst = t1[:, (c * WT + wt) * 512 : (c * WT + wt + 1) * 512]
                evac(dst, ps1[:])

            def stage2_group(s, c, ot, out4, pool=None):
                t1 = st(s, "t1", t_t1)
                CT = st(s, "CT", t_CT)
                if pool is None:
                    ps2 = ps2p.tile([128, 512], FP, tag="ps2", name="ps2")
                else:
                    ps2 = pool.tile([128, 512], FP, tag="ps1", name="ps1")
                for wt in range(WT):
                    lhsT2 = t1[
                        :,
                        (c * WT + wt) * 512 + ot * 128 : (c * WT + wt) * 512
                        + (ot + 1) * 128,
                    ]
                    nc.tensor.matmul(
                        ps2[:],
                        lhsT2,
                        CT[:, wt * 512 : (wt + 1) * 512],
                        start=(wt == 0),
                        stop=(wt == WT - 1),
                    )
                evac(out4[:, ot, :, c], ps2[:])

            def emit_out(s):
                outt = state[(s, "outt")]
                nc.sync.dma_start(
                    out_d[s]
                    .rearrange("(t p) w c -> t p (w c)", p=128)
                    .transpose([1, 0, 2]),
                    outt[:].rearrange("p (t x) -> p t x", t=HT),
                )

            # ---------------- software-pipelined emission ----------------
            pre_load(0)
            warm_burst(10)
            pre_early(0)
            pre_mid(0)
            warm_burst(10)
            pre_late(0)
            warm_burst(10)
            pre_mh(0)
            warm_burst(14)

            for s in range(bpc):
                nxt = s + 1 if s + 1 < bpc else None
                if nxt is not None:
                    pre_load(nxt)

                outt = st(s, "outt", t_outt)
                out4 = outt[:].rearrange("p (t w c) -> p t w c", t=HT, w=512)

                g1 = [(c, wt) for c in range(C) for wt in range(WT)]
                for i, (c, wt) in enumerate(g1):
                    stage1_group(s, c, wt, pool=(None if i % 2 == 0 else ps2p))
                    if i == 1 and nxt is not None:
                        pre_early(nxt)
                    if i == 4 and nxt is not None:
                        pre_mid(nxt)
                    if i == 6 and nxt is not None:
                        pre_late(nxt, which=(0,))

                outd3 = out_d[s].rearrange("(t p) w c -> t p (w c)", p=128)
                g2 = [(ot, c) for ot in range(HT) for c in range(C)]
                for i, (ot, c) in enumerate(g2):
                    stage2_group(s, c, ot, out4, pool=(None if i % 2 == 0 else ps1p))
                    if i == 0 and nxt is not None:
                        pre_late(nxt, which=(1,))
                    if i == 2 and nxt is not None:
                        pre_mh(nxt, chans=(0,))
                    if i == 4 and nxt is not None:
                        pre_mh(nxt, chans=(1,))
                    if i == 6 and nxt is not None:
                        pre_mh(nxt, chans=(2,))
                    if i % C == C - 1:
                        nc.sync.dma_start(
                            outd3[ot],
                            outt[:, ot * 512 * C : (ot + 1) * 512 * C],
                        )

    nc.compile()
    return nc


def make_consts() -> dict[str, np.ndarray]:
    import ml_dtypes

    iota_f = np.broadcast_to(np.arange(512, dtype=np.float32), (128, 512)).copy()
    p = np.arange(128, dtype=np.float32)
    pidx = np.stack([p + 128 * t for t in range(HT)], axis=1).astype(np.float32)
    tvals = np.broadcast_to(
        np.arange(HT, dtype=np.float32)[None, :], (128, HT)
    ).astype(np.float32)
    pvals = np.broadcast_to(p[:, None], (128, HT)).astype(np.float32)
    ones4 = np.ones((128, HT), np.float32)
    tp_h = np.concatenate([ones4, tvals, pvals], axis=1).astype(ml_dtypes.bfloat16)
    return {"iota_f": iota_f, "pidx": pidx, "tp_h": tp_h}


_NC_CACHE: dict[int, bass.Bass] = {}


def _get_nc(bpc: int = BPC) -> bass.Bass:
    if bpc not in _NC_CACHE:
        _NC_CACHE[bpc] = build(bpc)
    return _NC_CACHE[bpc]


def run(mask: np.ndarray, image: np.ndarray, trace: bool = False, **kwargs):
    """Run on 8 cores; returns (out [B,H,W,C], BassKernelResults)."""
    from concourse.bass_utils import run_bass_kernel_spmd

    nc = _get_nc(BPC)
    consts = make_consts()
    mask = np.ascontiguousarray(mask, dtype=np.float32)
    image = np.ascontiguousarray(image, dtype=np.float32)
    in_maps = []
    for i in range(N_CORES):
        m = {
            "mask": mask[i * BPC : (i + 1) * BPC],
            "image": image[i * BPC : (i + 1) * BPC],
        }
        m.update(consts)
        in_maps.append(m)
    res = run_bass_kernel_spmd(nc, in_maps, list(range(N_CORES)), trace=trace, **kwargs)
    out = np.concatenate([res.results[i]["out"] for i in range(N_CORES)], axis=0)
    return out, res


def kernel(mask: np.ndarray, image: np.ndarray) -> np.ndarray:
    out, _ = run(mask, image)
    return out.astype(np.float32)


# revision 45
# speedup vs baseline: 1.0383x; 1.0114x over previous
"""Trainium2 Bass kernel for nn_MergeZoom: per-sample mask bbox + crop + bilinear resize.

Algorithm (per sample, all on-device):
  mb   = (mask >= 0.5)
  rows/cols nonzero -> bbox (first,last per axis) via exact count/weighted-sum trick
  out  = R @ (mb * image) @ C^T  where R/C are bilinear "tent" matrices built on-chip.
  Tents are built NEGATED (min(|src-p|,1) - 1 = -relu(1-|src-p|)); the sign cancels
  across the two interpolation matmul stages.

Perf notes (176us -> ~136us):
  - PE warmup bursts at kernel start keep the HAM clock-gate at 2.4 GHz (cold MMs
    run at 1.2 GHz: 427ns vs 216ns for an N=512 bf16 matmul).
  - Masked image stored planar [p, (c, t, w)] so stage-1 lhsT slabs are contiguous
    (enables FWL weight-load overlap; strided weights serialize LDW with the MM).
  - Software-pipelined across samples: preprocessing of sample s+1 (mask stats on
    Scalar w/ accum_out, bbox chain batched [128,2] on DVE, tents split
    Scalar(A)/DVE(B)) overlaps the interp matmuls of sample s on PE.
  - Row tents positive (Scalar Relu), col tents negated (DVE min-sub); the sign is
    fixed for free in the stage-2 PSUM evacuation (scale=-1).
  - PSUM: both pools' rings are shared by both stages (7 tiles in flight) to hide
    evacuation latency; output DMA streams per 128-row chunk.
  - GpSimd deliberately unused: its tensor ops are ~10-20x slower than DVE and
    contend for the shared SBUF port.

Sharding: pure data-parallel, 4 samples per core across 8 cores.
"""

import numpy as np

import concourse.bass as bass
import concourse.tile as tile
from concourse import bacc, mybir

B = 32
N_CORES = 8
BPC = B // N_CORES  # samples per core
H = W = 512
C = 3
HT = H // 128  # 4 h-chunks of 128 partitions
WT = W // 128

FP = mybir.dt.float32
BF = mybir.dt.bfloat16
AX = mybir.AxisListType.X
OP = mybir.AluOpType
AF = mybir.ActivationFunctionType

N_WARMUP = 40


def build(bpc: int = BPC) -> bass.Bass:
    nc = bacc.Bacc()
    mask_d = nc.declare_dram_parameter("mask", [bpc, H, W, 1], FP, isOutput=False)
    img_d = nc.declare_dram_parameter("image", [bpc, H, W, C], FP, isOutput=False)
    iota_d = nc.declare_dram_parameter("iota_f", [128, 512], FP, isOutput=False)
    pidx_d = nc.declare_dram_parameter("pidx", [128, HT], FP, isOutput=False)
    tp_d = nc.declare_dram_parameter("tp_h", [128, 3 * HT], BF, isOutput=False)
    out_d = nc.declare_dram_parameter("out", [bpc, H, W, C], FP, isOutput=True)

    with tile.TileContext(nc) as tc:
        with (
            tc.tile_pool(name="consts", bufs=1) as cpool,
            tc.tile_pool(name="io", bufs=2) as iopool,
            tc.tile_pool(name="work", bufs=1) as wk,
            tc.tile_pool(name="small", bufs=2) as sm,
            tc.tile_pool(name="ps1", bufs=4, space="PSUM") as ps1p,
            tc.tile_pool(name="ps2", bufs=3, space="PSUM") as ps2p,
        ):
            iota = cpool.tile([128, 512], FP)
            nc.sync.dma_start(iota[:], iota_d[:])
            pidx = cpool.tile([128, HT], FP)
            nc.sync.dma_start(pidx[:], pidx_d[:])
            tp = cpool.tile([128, 3 * HT], BF)
            nc.sync.dma_start(tp[:], tp_d[:])
            onesh = cpool.tile([128, 128], BF)
            nc.vector.memset(onesh[:], 1.0)
            warm_rhs = cpool.tile([128, 512], BF)
            nc.vector.memset(warm_rhs[:], 0.001)
            negp = cpool.tile([128, HT], FP)
            nc.vector.tensor_scalar(negp[:], pidx[:], -1.0, None, OP.mult)
            neghalf = cpool.tile([128, 1], FP)
            nc.vector.memset(neghalf[:], -0.5)

            # persistent tiles (explicit double-buffer via tags)
            def t_msk(s):
                return sm.tile([128, HT * 512], FP, tag="msk", bufs=1, name="msk")

            def t_img(s):
                return iopool.tile([128, HT * 512 * C], FP, tag="img", bufs=2, name="img")

            def t_outt(s):
                return iopool.tile([128, HT * 512 * C], FP, tag="outt", bufs=2, name="outt")

            def t_mbh(s):
                return sm.tile([128, HT * 512], BF, tag="mbh", bufs=1, name="mbh")

            def t_Mh(s):
                return wk.tile([128, C * HT * 512], BF, tag="Mh", bufs=2, name="Mh")

            def t_t1(s):
                return wk.tile([128, C * WT * 512], BF, tag="t1", bufs=2, name="t1")

            def t_RT(s):
                return wk.tile([128, HT * 512], BF, tag="RT", bufs=2, name="RT")

            def t_CT(s):
                return wk.tile([128, WT * 512], BF, tag="CT", bufs=2, name="CT")

            # small per-sample state
            def t_r4(s):
                return sm.tile([128, HT], FP, tag="r4", bufs=2, name="r4")

            def t_NS(s):
                return sm.tile([128, 8], FP, tag="NS", bufs=2, name="NS")

            def t_sc(s):
                return sm.tile([128, 16], FP, tag="sc", bufs=2, name="sc")

            def t_src(s, o):
                return sm.tile([128, 512], FP, tag=f"src{o}", bufs=2, name=f"src{o}")

            # tile-state caches so helper phases share handles per sample
            state: dict = {}

            def st(s, key, mk):
                k = (s, key)
                if k not in state:
                    state[k] = mk(s)
                return state[k]

            # ---------------- warmup: keep PE HAM clock-gate open ----------------
            def warm_burst(n):
                warm_ps = ps2p.tile([128, 512], FP, tag="pscols", name="pscols", bufs=1)
                for _ in range(n):
                    nc.tensor.matmul(
                        warm_ps[:], onesh[:], warm_rhs[:], start=True, stop=True
                    )

            warm_burst(12)

            # ---------------- emission helpers ----------------
            def pre_load(s):
                msk = st(s, "msk", t_msk)
                mskd = mask_d[s].rearrange("(t p) w one -> t p (w one)", p=128)
                for t in range(HT):
                    nc.sync.dma_start(
                        msk[:, t * 512 : (t + 1) * 512], mskd[t]
                    )
                img = st(s, "img", t_img)
                nc.sync.dma_start(
                    img[:].rearrange("p (t x) -> p t x", t=HT),
                    img_d[s]
                    .rearrange("(t p) w c -> t p (w c)", p=128)
                    .transpose([1, 0, 2]),
                )

            def pre_early(s):
                # binarize + row-sum accum per chunk (DVE); col-sum MMs (PE)
                msk = st(s, "msk", t_msk)
                mbh = st(s, "mbh", t_mbh)
                r4 = st(s, "r4", t_r4)
                pscols = ps2p.tile([128, 512], FP, tag="pscols", name="pscols", bufs=1)
                state[(s, "pscols")] = pscols
                for t in range(HT):
                    nc.scalar.activation(
                        mbh[:, t * 512 : (t + 1) * 512],
                        msk[:, t * 512 : (t + 1) * 512],
                        AF.Relu, bias=neghalf[:], scale=1.0,
                        accum_out=r4[:, t : t + 1],
                    )
                    nc.tensor.matmul(
                        pscols[:],
                        onesh[:],
                        mbh[:, t * 512 : (t + 1) * 512],
                        start=(t == 0),
                        stop=(t == HT - 1),
                    )
                rwh = sm.tile([128, 3 * HT], BF, tag="rwh", bufs=2, name="rwh")
                state[(s, "rwh")] = rwh
                nc.vector.tensor_scalar(rwh[:, 0:HT], r4[:], 0.0, None, OP.is_gt)
                nc.vector.tensor_tensor(
                    rwh[:, HT : 2 * HT], rwh[:, 0:HT], tp[:, HT : 2 * HT], OP.mult
                )
                nc.vector.tensor_tensor(
                    rwh[:, 2 * HT : 3 * HT], rwh[:, 0:HT], tp[:, 2 * HT : 3 * HT], OP.mult
                )


            def pre_mid(s):
                # bbox stats: NS = [Nr, Nc, Sr(_t), Sc, Sr_p]
                pscols = state[(s, "pscols")]
                NS = st(s, "NS", t_NS)
                colscr = sm.tile([128, 512], FP, tag="colscr", bufs=1, name="colscr")
                redscr = sm.tile([128, 3 * HT], FP, tag="redscr", bufs=2, name="redscr")
                nc.scalar.activation(
                    colscr[:], pscols[:], AF.Sign, accum_out=NS[:, 1:2]
                )
                colt = sm.tile([128, 512], FP, tag="colt", bufs=2, name="colt")
                nc.vector.scalar_tensor_tensor(
                    colt[:], pscols[:], 0.0, iota[:], OP.is_gt, OP.mult,
                    accum_out=NS[:, 3:4],
                )
                rwh = state[(s, "rwh")]
                nc.tensor.matmul(
                    pscols[:, 0 : 3 * HT], onesh[:], rwh[:], start=True, stop=True,
                    skip_group_check=True,
                )
                nc.scalar.activation(
                    redscr[:, 0:HT], pscols[:, 0:HT], AF.Copy,
                    accum_out=NS[:, 0:1],
                )
                nc.scalar.activation(
                    redscr[:, HT : 2 * HT], pscols[:, HT : 2 * HT], AF.Copy,
                    accum_out=NS[:, 2:3],
                )
                nc.scalar.activation(
                    redscr[:, 2 * HT : 3 * HT], pscols[:, 2 * HT : 3 * HT], AF.Copy,
                    accum_out=NS[:, 4:5],
                )
                # Sr = 128*Sr_t + Sr_p
                nc.scalar.activation(
                    NS[:, 2:3], NS[:, 2:3], AF.Identity, scale=128.0, bias=NS[:, 4:5]
                )
                # batched bbox chain over both axes: [Nr, Nc] = NS[:,0:2], [Sr, Sc] = NS[:,2:4]
                # sc: 0:2 recip | 2:4 mean | 4:6 hw | 6:8 first | 8:10 last(hi1)
                #     | 10:12 a | 12:14 b | 14:16 lo
                sc = st(s, "sc", t_sc)
                nc.vector.reciprocal(sc[:, 0:2], NS[:, 0:2])
                nc.vector.tensor_tensor(sc[:, 2:4], NS[:, 2:4], sc[:, 0:2], OP.mult)
                nc.vector.tensor_scalar(
                    sc[:, 4:6], NS[:, 0:2], -1.0, 0.5, OP.add, OP.mult
                )
                nc.vector.tensor_tensor(sc[:, 6:8], sc[:, 2:4], sc[:, 4:6], OP.subtract)
                nc.vector.tensor_tensor(sc[:, 8:10], sc[:, 2:4], sc[:, 4:6], OP.add)
                nc.vector.tensor_scalar(
                    sc[:, 10:12], NS[:, 0:2], 1.0, 1.0 / 512.0, OP.add, OP.mult
                )
                nc.vector.tensor_scalar(
                    sc[:, 12:14], sc[:, 10:12], 0.5, -1.5, OP.mult, OP.add
                )
                nc.vector.tensor_tensor(sc[:, 12:14], sc[:, 12:14], sc[:, 6:8], OP.add)
                nc.vector.tensor_scalar(sc[:, 14:16], sc[:, 6:8], -1.0, None, OP.add)

            def pre_late(s, which=(0, 1)):
                sc = st(s, "sc", t_sc)
                axes = [(0, "RT", t_RT), (1, "CT", t_CT)]
                for o, key, mk in (axes[i] for i in which):
                    mat = st(s, key, mk)
                    src = st(s, f"src{o}", lambda s_: t_src(s_, o))
                    nc.vector.tensor_scalar(
                        src[:], iota[:], sc[:, 10 + o : 11 + o], sc[:, 12 + o : 13 + o],
                        OP.mult, OP.add,
                    )
                    nc.vector.tensor_scalar(
                        src[:], src[:], sc[:, 14 + o : 15 + o], sc[:, 8 + o : 9 + o],
                        OP.max, OP.min,
                    )
                    tmp4 = sm.tile(
                        [128, HT * 512], BF, tag=f"tent{o}", bufs=2, name=f"tent{o}"
                    )
                    for t in range(HT):
                        nc.scalar.activation(
                            tmp4[:, t * 512 : (t + 1) * 512], src[:], AF.Abs,
                            bias=negp[:, t : t + 1], scale=1.0,
                        )
                        if o == 0:
                            nc.vector.tensor_scalar(
                                mat[:, t * 512 : (t + 1) * 512],
                                tmp4[:, t * 512 : (t + 1) * 512],
                                1.0, 1.0, OP.min, OP.subtract,
                            )
                    if o == 1:
                        nc.vector.tensor_scalar(
                            mat[:], tmp4[:], 1.0, 1.0, OP.min, OP.subtract
                        )

            def pre_mh(s, chans=(0, 1, 2), thalf=None):
                # masked image, planar [p, (c, t, w)] (DVE, fused binarize+mult)
                msk = st(s, "msk", t_msk)
                img = st(s, "img", t_img)
                Mh = st(s, "Mh", t_Mh)
                img4 = img[:].rearrange("p (t w c) -> p t w c", t=HT, w=512)
                Mh4 = Mh[:].rearrange("p (c t w) -> p c t w", c=C, t=HT)
                msk3 = msk[:].rearrange("p (t w) -> p t w", t=HT)
                tsl = slice(None) if thalf is None else slice(
                    thalf * (HT // 2), (thalf + 1) * (HT // 2)
                )
                for c in chans:
                    nc.vector.scalar_tensor_tensor(
                        Mh4[:, c, tsl], msk3[:, tsl], 0.5, img4[:, tsl, :, c],
                        OP.is_ge, OP.mult,
                    )

            evac_cp = [0]

            def evac(dst, ps, negate=False, dve_mod=3):
                if evac_cp[0] % dve_mod == 0:
                    if negate:
                        nc.vector.tensor_scalar(dst, ps, -1.0, None, OP.mult)
                    else:
                        nc.vector.tensor_copy(dst, ps)
                else:
                    if negate:
                        nc.scalar.activation(dst, ps, AF.Identity, scale=-1.0)
                    else:
                        nc.scalar.copy(dst, ps)
                evac_cp[0] += 1

            def stage1_group(s, c, wt, pool=None):
                Mh = st(s, "Mh", t_Mh)
                RT = st(s, "RT", t_RT)
                t1 = st(s, "t1", t_t1)
                Mh4 = Mh[:].rearrange("p (c t w) -> p c t w", c=C, t=HT)
                if pool is None:
                    ps1 = ps1p.tile([128, 512], FP, tag="ps1", name="ps1")
                else:
                    ps1 = pool.tile([128, 512], FP, tag="ps2", name="ps2")
                for ht in range(HT):
                    lhsT = Mh4[:, c, ht, wt * 128 : (wt + 1) * 128]
                    nc.tensor.matmul(
                        ps1[:],
                        lhsT,
                        RT[:, ht * 512 : (ht + 1) * 512],
                        start=(ht == 0),
                        stop=(ht == HT - 1),
                    )
                dst = t1[:, (c * WT + wt) * 512 : (c * WT + wt + 1) * 512]
                evac(dst, ps1[:])

            def stage2_group(s, c, ot, out4, pool=None):
                t1 = st(s, "t1", t_t1)
                CT = st(s, "CT", t_CT)
                if pool is None:
                    ps2 = ps2p.tile([128, 512], FP, tag="ps2", name="ps2")
                else:
                    ps2 = pool.tile([128, 512], FP, tag="ps1", name="ps1")
                for wt in range(WT):
                    lhsT2 = t1[
                        :,
                        (c * WT + wt) * 512 + ot * 128 : (c * WT + wt) * 512
                        + (ot + 1) * 128,
                    ]
                    nc.tensor.matmul(
                        ps2[:],
                        lhsT2,
                        CT[:, wt * 512 : (wt + 1) * 512],
                        start=(wt == 0),
                        stop=(wt == WT - 1),
                    )
                evac(out4[:, ot, :, c], ps2[:])

            def emit_out(s):
                outt = state[(s, "outt")]
                nc.sync.dma_start(
                    out_d[s]
                    .rearrange("(t p) w c -> t p (w c)", p=128)
                    .transpose([1, 0, 2]),
                    outt[:].rearrange("p (t x) -> p t x", t=HT),
                )

            # ---------------- software-pipelined emission ----------------
            pre_load(0)
            warm_burst(10)
            pre_early(0)
            pre_mid(0)
            warm_burst(10)
            pre_late(0)
            warm_burst(10)
            pre_mh(0)
            warm_burst(14)

            for s in range(bpc):
                nxt = s + 1 if s + 1 < bpc else None
                if nxt is not None:
                    pre_load(nxt)

                outt = st(s, "outt", t_outt)
                out4 = outt[:].rearrange("p (t w c) -> p t w c", t=HT, w=512)

                g1 = [(c, wt) for c in range(C) for wt in range(WT)]
                for i, (c, wt) in enumerate(g1):
                    stage1_group(s, c, wt, pool=(None if i % 2 == 0 else ps2p))
                    if i == 1 and nxt is not None:
                        pre_early(nxt)
                    if i == 4 and nxt is not None:
                        pre_mid(nxt)
                    if i == 6 and nxt is not None:
                        pre_late(nxt, which=(0,))

                outd3 = out_d[s].rearrange("(t p) w c -> t p (w c)", p=128)
                g2 = [(ot, c) for ot in range(HT) for c in range(C)]
                for i, (ot, c) in enumerate(g2):
                    stage2_group(s, c, ot, out4, pool=(None if i % 2 == 0 else ps1p))
                    if i == 0 and nxt is not None:
                        pre_late(nxt, which=(1,))
                    if i == 2 and nxt is not None:
                        pre_mh(nxt, chans=(0,))
                    if i == 4 and nxt is not None:
                        pre_mh(nxt, chans=(1,))
                    if i == 6 and nxt is not None:
                        pre_mh(nxt, chans=(2,))
                    if i % C == C - 1:
                        nc.sync.dma_start(
                            outd3[ot],
                            outt[:, ot * 512 * C : (ot + 1) * 512 * C],
                        )

    nc.compile()
    return nc


def make_consts() -> dict[str, np.ndarray]:
    import ml_dtypes

    iota_f = np.broadcast_to(np.arange(512, dtype=np.float32), (128, 512)).copy()
    p = np.arange(128, dtype=np.float32)
    pidx = np.stack([p + 128 * t for t in range(HT)], axis=1).astype(np.float32)
    tvals = np.broadcast_to(
        np.arange(HT, dtype=np.float32)[None, :], (128, HT)
    ).astype(np.float32)
    pvals = np.broadcast_to(p[:, None], (128, HT)).astype(np.float32)
    ones4 = np.ones((128, HT), np.float32)
    tp_h = np.concatenate([ones4, tvals, pvals], axis=1).astype(ml_dtypes.bfloat16)
    return {"iota_f": iota_f, "pidx": pidx, "tp_h": tp_h}


_NC_CACHE: dict[int, bass.Bass] = {}


def _get_nc(bpc: int = BPC) -> bass.Bass:
    if bpc not in _NC_CACHE:
        _NC_CACHE[bpc] = build(bpc)
    return _NC_CACHE[bpc]


def run(mask: np.ndarray, image: np.ndarray, trace: bool = False, **kwargs):
    """Run on 8 cores; returns (out [B,H,W,C], BassKernelResults)."""
    from concourse.bass_utils import run_bass_kernel_spmd

    nc = _get_nc(BPC)
    consts = make_consts()
    mask = np.ascontiguousarray(mask, dtype=np.float32)
    image = np.ascontiguousarray(image, dtype=np.float32)
    in_maps = []
    for i in range(N_CORES):
        m = {
            "mask": mask[i * BPC : (i + 1) * BPC],
            "image": image[i * BPC : (i + 1) * BPC],
        }
        m.update(consts)
        in_maps.append(m)
    res = run_bass_kernel_spmd(nc, in_maps, list(range(N_CORES)), trace=trace, **kwargs)
    out = np.concatenate([res.results[i]["out"] for i in range(N_CORES)], axis=0)
    return out, res


def kernel(mask: np.ndarray, image: np.ndarray) -> np.ndarray:
    out, _ = run(mask, image)
    return out.astype(np.float32)
